# revision 21
# baseline (speedup 1.0000x reference)
"""Trainium2 Bass kernel for the DWA middle layer (moe_routing).

Math (factored form of the reference):
    t     = h_A @ V_flat^T                      # [B, N*R]
    s     = t * repeat(alpha, R, axis=1)        # [B, N*R]
    h_T   = s @ U_flat^T + h_A @ W_base^T + [alpha, 1] @ [bias_pool; b_base]
    out   = LayerNorm(h_A + gamma * h_T) * ln_scale + ln_bias

Sharding: data-parallel over the batch dim (32 rows per core, 8 cores).
Weight matrices are replicated but streamed as fp8e4 (the 2e-2 rel-err
gate leaves ~10x margin for 8-bit weights; V/W/bias are pre-scaled by
32 on the host to sit in e4m3's normal range, U stays at 1x because the
s operand already carries the 32x, and the 1/32 is folded into gamma).
That cuts the per-core HBM stream from 12.5MB to ~3.3MB.

All PE matmuls keep the (small) activations stationary and stream the
weight matrices as the moving operand at N=512.
"""

import os
from contextlib import ExitStack

import numpy as np

import concourse.bacc as bacc
import concourse.mybir as mybir
import concourse.tile as tile
from concourse import bass_utils, masks

F32 = mybir.dt.float32
BF16 = mybir.dt.bfloat16
FP8 = mybir.dt.float8e4

D = 1024          # d_A == d_B
B_CORE = 32       # batch rows per core
N_EXP = 64        # experts
R_RANK = 16       # rank per expert
N_CORES = 8
KT = D // 128     # 8 contraction tiles of 128
NH = D // 512     # 2 moving halves of 512
WSCALE = 32.0     # host-side scale on V/W/bias to center fp8e4m3

STAGE = os.environ.get("DWA_STAGE", "full")

_COMPILED = {}


def _build(stage="full", ln_triv=False):
    nc = bacc.Bacc("TRN2", debug=False, num_devices=N_CORES,
                   enable_partition_id=False)

    ha_d = nc.dram_tensor("ha", [B_CORE, D], F32, kind="ExternalInput")
    al_d = nc.dram_tensor("al", [B_CORE, N_EXP], F32, kind="ExternalInput")
    alr_d = nc.dram_tensor("alr", [B_CORE, 4 * KT * B_CORE], BF16,
                           kind="ExternalInput")
    # h_A^T in SBUF-native partition-major layout [128, KT*B_CORE], fp8
    xt_d = nc.dram_tensor("xt", [128, KT * B_CORE], FP8, kind="ExternalInput")
    # weights in SBUF-native partition-major layout [128, KT*1024], fp8
    vt_d = nc.dram_tensor("vt", [128, KT * D], FP8, kind="ExternalInput")
    ut_d = nc.dram_tensor("ut", [128, KT * D], FP8, kind="ExternalInput")
    wt_d = nc.dram_tensor("wt", [128, KT * D], FP8, kind="ExternalInput")
    # [alpha^T; 1] and [bias_pool; b_base] for the bias term, bf16
    alt_d = nc.dram_tensor("alt", [N_EXP + 1, B_CORE], BF16, kind="ExternalInput")
    bp_d = nc.dram_tensor("bp", [N_EXP + 1, D], BF16, kind="ExternalInput")
    lns_d = nc.dram_tensor("lns", [1, D], F32, kind="ExternalInput")
    lnb_d = nc.dram_tensor("lnb", [1, D], F32, kind="ExternalInput")
    gm_d = nc.dram_tensor("gm", [1, 1], F32, kind="ExternalInput")
    out_d = nc.dram_tensor("out", [B_CORE, D], F32, kind="ExternalOutput")

    with ExitStack() as ctx:
        tc = ctx.enter_context(tile.TileContext(nc))
        _emit(ctx, tc, stage, ln_triv, ha_d, al_d, alr_d, xt_d, vt_d, ut_d,
              wt_d, alt_d, bp_d, lns_d, lnb_d, gm_d, out_d)

    nc.compile()
    return nc


def _emit(ctx, tc, stage, ln_triv, ha_d, al_d, alr_d, xt_d, vt_d, ut_d,
          wt_d, alt_d, bp_d, lns_d, lnb_d, gm_d, out_d):
    nc = tc.nc
    MULT = mybir.AluOpType.mult
    ADD = mybir.AluOpType.add
    SQRT = mybir.ActivationFunctionType.Sqrt

    wpool = ctx.enter_context(tc.tile_pool(name="weights", bufs=1))
    sm = ctx.enter_context(tc.tile_pool(name="small", bufs=1))
    acc = ctx.enter_context(tc.tile_pool(name="acc", bufs=1, space="PSUM"))

    vt_sb = wpool.tile([128, KT * D], FP8, tag="vt")
    ut_sb = wpool.tile([128, KT * D], FP8, tag="ut")
    wt_sb = wpool.tile([128, KT * D], FP8, tag="wt")

    ha_sb = sm.tile([B_CORE, D], F32, tag="ha")
    al_sb = sm.tile([B_CORE, N_EXP], F32, tag="al")
    alr_sb = sm.tile([B_CORE, 4 * KT * B_CORE], BF16, tag="alr")
    xt_sb = sm.tile([128, KT * B_CORE], FP8, tag="xt")   # h_A^T tiles
    alt_sb = sm.tile([N_EXP + 1, B_CORE], BF16, tag="alt")  # [alpha^T; 1]
    bp_sb = sm.tile([N_EXP + 1, D], BF16, tag="bp")
    s_sb = sm.tile([B_CORE, D], F32, tag="s")
    st_sb = sm.tile([128, KT * B_CORE], FP8, tag="st")   # s^T tiles, fp8
    hpre_sb = sm.tile([B_CORE, D], F32, tag="hpre")
    sq_sb = sm.tile([B_CORE, D], F32, tag="sq")
    y_sb = sm.tile([B_CORE, D], F32, tag="y")
    t2_sb = sm.tile([B_CORE, D], F32, tag="t2")
    out_sb = sm.tile([B_CORE, D], F32, tag="out")
    lnsr_sb = sm.tile([B_CORE, D], F32, tag="lnsr")
    lnbr_sb = sm.tile([B_CORE, D], F32, tag="lnbr")
    gmc_sb = sm.tile([B_CORE, 1], F32, tag="gmc")
    sum_h = [sm.tile([B_CORE, 1], F32, tag=f"sumh{h}", name=f"sumh{h}")
             for h in range(NH)]
    ssq_h = [sm.tile([B_CORE, 1], F32, tag=f"ssqh{h}", name=f"ssqh{h}")
             for h in range(NH)]
    sum_c = sm.tile([B_CORE, 1], F32, tag="sumc")
    m_c = sm.tile([B_CORE, 1], F32, tag="mc")
    msq_c = sm.tile([B_CORE, 1], F32, tag="msqc")
    ssq_c = sm.tile([B_CORE, 1], F32, tag="ssqc")
    var_c = sm.tile([B_CORE, 1], F32, tag="varc")
    std_c = sm.tile([B_CORE, 1], F32, tag="stdc")
    istd_c = sm.tile([B_CORE, 1], F32, tag="istdc")
    nmi_c = sm.tile([B_CORE, 1], F32, tag="nmic")
    eps_c = sm.tile([B_CORE, 1], F32, tag="epsc")
    warm_c = sm.tile([B_CORE, 1], F32, tag="warmc")

    # ---- three parallel DMA streams, ordered by first consumption:
    # sync: xt then vt/wt a-halves; scalar: activations then vt/wt
    # b-halves; gpsimd (SWDGE): bias then the whole ut matrix ----
    nc.sync.dma_start(out=xt_sb[:], in_=xt_d.ap())
    nc.scalar.dma_start(out=vt_sb[:, 4 * D:], in_=vt_d.ap()[:, 4 * D:])
    nc.scalar.dma_start(out=alt_sb[:], in_=alt_d.ap())
    nc.scalar.dma_start(out=al_sb[:], in_=al_d.ap())
    nc.sync.dma_start(out=vt_sb[:, :4 * D], in_=vt_d.ap()[:, :4 * D])
    nc.sync.dma_start(out=wt_sb[:, :4 * D], in_=wt_d.ap()[:, :4 * D])
    nc.scalar.dma_start(out=wt_sb[:, 4 * D:], in_=wt_d.ap()[:, 4 * D:])
    nc.gpsimd.dma_start(out=gmc_sb[:], in_=gm_d.ap().broadcast_to([B_CORE, 1]))
    nc.gpsimd.dma_start(out=bp_sb[:], in_=bp_d.ap())
    nc.gpsimd.dma_start(out=ut_sb[:, :4 * D], in_=ut_d.ap()[:, :4 * D])
    nc.gpsimd.dma_start(out=ut_sb[:, 4 * D:], in_=ut_d.ap()[:, 4 * D:])
    nc.scalar.dma_start(out=ha_sb[:], in_=ha_d.ap())
    if not ln_triv:
        nc.sync.dma_start(out=lnsr_sb[:],
                          in_=lns_d.ap().broadcast_to([B_CORE, D]))
        nc.scalar.dma_start(out=lnbr_sb[:],
                            in_=lnb_d.ap().broadcast_to([B_CORE, D]))

    nc.vector.memset(eps_c[:], 1e-5)

    if stage == "loads":
        nc.vector.tensor_copy(out_sb[:], ha_sb[:])
        nc.sync.dma_start(out=out_d.ap(), in_=out_sb[:])
        return

    # ---- PE warm-up: junk matmuls flip the HAM clock gate to 2.4GHz
    # before the real (DMA-gated) matmuls arrive ----
    warm_sb = sm.tile([128, 512], FP8, tag="warm")
    warm_ps = acc.tile([128, 512], F32, tag="warmps")
    nc.vector.memset(warm_sb[:], 1.0)
    for w in range(20):
        nc.tensor.matmul(warm_ps[:], warm_sb[:, :128], warm_sb[:],
                         start=True, stop=True)

    # ---- column-packed matmuls: the four M=32 accumulation chains
    # (t h=0, t h=1, h_T h=0, h_T h=1) run in distinct 32-column strips
    # of the PE array (tile_position), so they execute concurrently.
    # One [128, 512] PSUM bank holds all four as partition bands. ----
    tw = acc.tile([128, 512], F32, tag="tw")
    bnd = [tw[32 * c:32 * (c + 1), :] for c in range(4)]

    for h in range(NH):
        nc.tensor.matmul(bnd[2 + h], alt_sb[:],
                         bp_sb[:, 512 * h:512 * (h + 1)],
                         start=True, stop=False, tile_position=(0, 64 + 32 * h),
                         skip_group_check=True)
    for i in range(KT):
        xt_i = xt_sb[:, B_CORE * i:B_CORE * (i + 1)]
        for h in range(NH):
            nc.tensor.matmul(
                bnd[h], xt_i,
                vt_sb[:, D * i + 512 * h:D * i + 512 * (h + 1)],
                start=(i == 0), stop=(i == KT - 1),
                tile_position=(0, 32 * h), skip_group_check=True,
            )

    # s = t * repeat(alpha, R)  (reads the t bands at shifted partitions)
    for h in range(NH):
        o3 = s_sb[:, 512 * h:512 * (h + 1)].rearrange(
            "p (n r) -> p n r", r=R_RANK)
        i3 = bnd[h].rearrange("p (n r) -> p n r", r=R_RANK)
        a3 = al_sb[:, 32 * h:32 * (h + 1)].unsqueeze(-1).broadcast_to(
            [B_CORE, 32, R_RANK])
        nc.vector.tensor_mul(o3, i3, a3)
    for i in range(KT):
        xt_i = xt_sb[:, B_CORE * i:B_CORE * (i + 1)]
        for h in range(NH):
            nc.tensor.matmul(
                bnd[2 + h], xt_i,
                wt_sb[:, D * i + 512 * h:D * i + 512 * (h + 1)],
                start=False, stop=False, tile_position=(0, 64 + 32 * h),
                skip_group_check=True,
            )
    # preload both ACT tables (Square, Sqrt) before the LN tail needs them
    nc.scalar.activation(warm_c[:], eps_c[:],
                         mybir.ActivationFunctionType.Square)
    nc.scalar.activation(warm_c[:], eps_c[:], SQRT, bias=eps_c[:], scale=1.0)
    nc.scalar.activation(warm_c[:], eps_c[:],
                         mybir.ActivationFunctionType.Identity)

    if stage == "t":
        nc.sync.dma_start(out=out_d.ap(), in_=s_sb[:])
        return

    # ---- s^T tiles: one DVE 32x32 block-transpose, then 4 strided
    # partition-shifted copy-casts gather the blocks into fp8 [128, .] ----
    stt_sb = sm.tile([B_CORE, D], F32, tag="stt")
    nc.vector.transpose(stt_sb[:], s_sb[:])
    for b in range(4):
        src_ap = stt_sb[:].rearrange(
            "p (j m) -> p j m", j=KT)[:, :, 32 * b:32 * b + B_CORE]
        dst = st_sb[32 * b:32 * (b + 1), :].rearrange(
            "p (j m) -> p j m", m=B_CORE)
        nc.vector.tensor_copy(dst, src_ap)

    # ---- h_T += s @ U^T (same column strips / bands as the W chain) ----
    for j in range(KT):
        st_j = st_sb[:, B_CORE * j:B_CORE * (j + 1)]
        for h in range(NH):
            nc.tensor.matmul(
                bnd[2 + h], st_j,
                ut_sb[:, D * j + 512 * h:D * j + 512 * (h + 1)],
                start=False, stop=(j == KT - 1),
                tile_position=(0, 64 + 32 * h), skip_group_check=True,
            )
    for h in range(NH):
        sl = slice(512 * h, 512 * (h + 1))
        # h_pre = (gamma/WSCALE) * h_T + h_A, with row-sums for the mean
        nc.vector.scalar_tensor_tensor(
            out=hpre_sb[:, sl], in0=bnd[2 + h], scalar=gmc_sb[:],
            in1=ha_sb[:, sl], op0=MULT, op1=ADD,
            accum_out=sum_h[h][:])
        # row-sums of squares on the Scalar engine (parallel to DVE)
        nc.scalar.activation(sq_sb[:, sl], hpre_sb[:, sl],
                             mybir.ActivationFunctionType.Square,
                             accum_out=ssq_h[h][:])

    if stage == "h":
        nc.sync.dma_start(out=out_d.ap(), in_=hpre_sb[:])
        return

    # ---- LayerNorm via E[x^2] - E[x]^2; istd in one ACT Rsqrt ----
    nc.vector.tensor_add(sum_c[:], sum_h[0][:], sum_h[1][:])
    nc.vector.tensor_add(ssq_c[:], ssq_h[0][:], ssq_h[1][:])
    nc.scalar.mul(m_c[:], sum_c[:], 1.0 / D)
    nc.vector.tensor_mul(msq_c[:], m_c[:], m_c[:])
    nc.vector.scalar_tensor_tensor(
        out=var_c[:], in0=ssq_c[:], scalar=1.0 / D, in1=msq_c[:],
        op0=MULT, op1=mybir.AluOpType.subtract)
    nc.scalar.activation(std_c[:], var_c[:], SQRT, bias=eps_c[:], scale=1.0)
    nc.vector.reciprocal(istd_c[:], std_c[:])
    nc.scalar.mul(nmi_c[:], m_c[:], -1.0)
    nc.vector.tensor_mul(nmi_c[:], nmi_c[:], istd_c[:])
    # norm = hpre*istd - m*istd (ACT), out = norm*lns + lnb (DVE)
    for h in range(NH):
        sl = slice(512 * h, 512 * (h + 1))
        nc.scalar.activation(y_sb[:, sl], hpre_sb[:, sl],
                             mybir.ActivationFunctionType.Identity,
                             bias=nmi_c[:], scale=istd_c[:])
        if ln_triv:
            nc.sync.dma_start(out=out_d.ap()[:, sl], in_=y_sb[:, sl])
        else:
            nc.vector.tensor_mul(t2_sb[:, sl], y_sb[:, sl], lnsr_sb[:, sl])
            nc.vector.tensor_add(out_sb[:, sl], t2_sb[:, sl], lnbr_sb[:, sl])
            nc.sync.dma_start(out=out_d.ap()[:, sl], in_=out_sb[:, sl])


def _to_sbuf_layout(a):
    """[KT*128, F] logical -> [128, KT*F] partition-major."""
    f = a.shape[1]
    return np.ascontiguousarray(
        a.reshape(KT, 128, f).transpose(1, 0, 2).reshape(128, KT * f))


_FP8 = mybir.dt.np(FP8)
_BF16 = mybir.dt.np(BF16)


def _prep_in_maps(inputs):
    def f32c(x):
        return np.ascontiguousarray(np.asarray(x, dtype=np.float32))

    h_a = f32c(inputs["h_A"])
    alpha = f32c(inputs["alpha"])
    pool = np.asarray(inputs["pool_vectors"], dtype=np.float32)
    w_base = np.asarray(inputs["W_base"], dtype=np.float32)

    # pool_vectors rows: [U_n (D*R) | V_n (R*D) | bias_n (D)]
    u = pool[:, :D * R_RANK].reshape(N_EXP, D, R_RANK)
    v = pool[:, D * R_RANK:2 * D * R_RANK].reshape(N_EXP, R_RANK, D)
    bias_pool = pool[:, 2 * D * R_RANK:]                    # [64, D]
    bb = np.asarray(inputs["b_base"], dtype=np.float32).reshape(1, D)
    bp = np.ascontiguousarray(
        (np.concatenate([bias_pool, bb], axis=0) * WSCALE).astype(_BF16))
    ut = np.ascontiguousarray(_to_sbuf_layout(
        u.transpose(0, 2, 1).reshape(N_EXP * R_RANK, D)).astype(_FP8))
    vt = np.ascontiguousarray(_to_sbuf_layout(
        v.reshape(N_EXP * R_RANK, D).T * WSCALE).astype(_FP8))
    wt = np.ascontiguousarray(
        _to_sbuf_layout(w_base.T * WSCALE).astype(_FP8))
    lns = f32c(inputs["ln_scale"]).reshape(1, D)
    lnb = f32c(inputs["ln_bias"]).reshape(1, D)
    gm = f32c(inputs["gamma"]).reshape(1, 1) / WSCALE

    in_maps = []
    for k in range(N_CORES):
        rows = slice(B_CORE * k, B_CORE * (k + 1))
        ha_k = f32c(h_a[rows])
        al_k = f32c(alpha[rows])
        xt_k = np.ascontiguousarray(_to_sbuf_layout(ha_k.T).astype(_FP8))
        alt_k = np.ascontiguousarray(np.concatenate(
            [al_k.T, np.ones((1, B_CORE), np.float32)], axis=0).astype(_BF16))
        p = np.arange(B_CORE)[:, None]
        col = np.arange(4 * KT * B_CORE)[None, :]
        b_i, jm = col // (KT * B_CORE), col % (KT * B_CORE)
        exp_idx = (128 * (jm // B_CORE) + 32 * b_i + p) // R_RANK
        m_idx = np.broadcast_to(jm % B_CORE, exp_idx.shape)
        alr_k = np.ascontiguousarray(al_k[m_idx, exp_idx].astype(_BF16))
        in_maps.append({
            "ha": ha_k, "al": al_k, "alr": alr_k, "xt": xt_k, "alt": alt_k,
            "vt": vt, "ut": ut, "wt": wt, "bp": bp,
            "lns": lns, "lnb": lnb, "gm": gm,
        })
    return in_maps


def get_compiled(stage=None, ln_triv=False):
    key = (stage or STAGE, ln_triv)
    if key not in _COMPILED:
        _COMPILED[key] = _build(*key)
    return _COMPILED[key]


def _ln_trivial(inputs):
    return (np.allclose(np.asarray(inputs["ln_scale"], np.float32), 1.0)
            and np.allclose(np.asarray(inputs["ln_bias"], np.float32), 0.0))


def kernel(**inputs):
    nc = get_compiled(ln_triv=_ln_trivial(inputs))
    in_maps = _prep_in_maps(inputs)
    res = bass_utils.run_bass_kernel_spmd(
        nc, in_maps, core_ids=list(range(N_CORES)))
    return np.concatenate([r["out"] for r in res.results], axis=0)


# revision 22
# speedup vs baseline: 1.1428x; 1.1428x over previous
"""Trainium2 Bass kernel for the DWA middle layer (moe_routing).

Math (factored form of the reference):
    t     = h_A @ V_flat^T                      # [B, N*R]
    s     = t * repeat(alpha, R, axis=1)        # [B, N*R]
    h_T   = s @ U_flat^T + h_A @ W_base^T + [alpha, 1] @ [bias_pool; b_base]
    out   = LayerNorm(h_A + gamma * h_T) * ln_scale + ln_bias

Sharding: data-parallel over the batch dim (32 rows per core, 8 cores).
Weight matrices are replicated but streamed as fp8e4 (the 2e-2 rel-err
gate leaves ~10x margin for 8-bit weights; V/W/bias are pre-scaled by
32 on the host to sit in e4m3's normal range, U stays at 1x because the
s operand already carries the 32x, and the 1/32 is folded into gamma).
That cuts the per-core HBM stream from 12.5MB to ~3.3MB.

All PE matmuls keep the (small) activations stationary and stream the
weight matrices as the moving operand at N=512.
"""

import os
from contextlib import ExitStack

import numpy as np

import concourse.bacc as bacc
import concourse.mybir as mybir
import concourse.tile as tile
from concourse import bass_utils, masks

F32 = mybir.dt.float32
BF16 = mybir.dt.bfloat16
FP8 = mybir.dt.float8e4

D = 1024          # d_A == d_B
B_CORE = 32       # batch rows per core
N_EXP = 64        # experts
R_RANK = 16       # rank per expert
N_CORES = 8
KT = D // 128     # 8 contraction tiles of 128
NH = D // 512     # 2 moving halves of 512
WSCALE = 32.0     # host-side scale on V/W/bias to center fp8e4m3

STAGE = os.environ.get("DWA_STAGE", "full")

_COMPILED = {}


def _build(stage="full", ln_triv=False):
    nc = bacc.Bacc("TRN2", debug=False, num_devices=N_CORES,
                   enable_partition_id=False)

    ha_d = nc.dram_tensor("ha", [B_CORE, D], F32, kind="ExternalInput")
    al_d = nc.dram_tensor("al", [B_CORE, N_EXP], F32, kind="ExternalInput")
    alr_d = nc.dram_tensor("alr", [B_CORE, 4 * KT * B_CORE], BF16,
                           kind="ExternalInput")
    # h_A^T in SBUF-native partition-major layout [128, KT*B_CORE], fp8
    xt_d = nc.dram_tensor("xt", [128, KT * B_CORE], FP8, kind="ExternalInput")
    # weights in SBUF-native partition-major layout [128, KT*1024], fp8
    vt_d = nc.dram_tensor("vt", [128, KT * D], FP8, kind="ExternalInput")
    ut_d = nc.dram_tensor("ut", [128, KT * D], FP8, kind="ExternalInput")
    wt_d = nc.dram_tensor("wt", [128, KT * D], FP8, kind="ExternalInput")
    # [alpha^T; 1] and [bias_pool; b_base] for the bias term, bf16
    alt_d = nc.dram_tensor("alt", [N_EXP + 1, B_CORE], BF16, kind="ExternalInput")
    bp_d = nc.dram_tensor("bp", [N_EXP + 1, D], BF16, kind="ExternalInput")
    lns_d = nc.dram_tensor("lns", [1, D], F32, kind="ExternalInput")
    lnb_d = nc.dram_tensor("lnb", [1, D], F32, kind="ExternalInput")
    gm_d = nc.dram_tensor("gm", [1, 1], F32, kind="ExternalInput")
    out_d = nc.dram_tensor("out", [B_CORE, D], F32, kind="ExternalOutput")

    with ExitStack() as ctx:
        tc = ctx.enter_context(tile.TileContext(nc))
        _emit(ctx, tc, stage, ln_triv, ha_d, al_d, alr_d, xt_d, vt_d, ut_d,
              wt_d, alt_d, bp_d, lns_d, lnb_d, gm_d, out_d)

    nc.compile()
    return nc


def _emit(ctx, tc, stage, ln_triv, ha_d, al_d, alr_d, xt_d, vt_d, ut_d,
          wt_d, alt_d, bp_d, lns_d, lnb_d, gm_d, out_d):
    nc = tc.nc
    MULT = mybir.AluOpType.mult
    ADD = mybir.AluOpType.add
    SQRT = mybir.ActivationFunctionType.Sqrt

    wpool = ctx.enter_context(tc.tile_pool(name="weights", bufs=1))
    sm = ctx.enter_context(tc.tile_pool(name="small", bufs=1))
    acc = ctx.enter_context(tc.tile_pool(name="acc", bufs=1, space="PSUM"))

    vt_sb = wpool.tile([128, KT * D], FP8, tag="vt")
    ut_sb = wpool.tile([128, KT * D], FP8, tag="ut")
    wt_sb = wpool.tile([128, KT * D], FP8, tag="wt")

    ha_sb = sm.tile([B_CORE, D], F32, tag="ha")
    al_sb = sm.tile([B_CORE, N_EXP], F32, tag="al")
    alr_sb = sm.tile([B_CORE, 4 * KT * B_CORE], BF16, tag="alr")
    xt_sb = sm.tile([128, KT * B_CORE], FP8, tag="xt")   # h_A^T tiles
    alt_sb = sm.tile([N_EXP + 1, B_CORE], BF16, tag="alt")  # [alpha^T; 1]
    bp_sb = sm.tile([N_EXP + 1, D], BF16, tag="bp")
    s_sb = sm.tile([B_CORE, D], BF16, tag="s")
    st_sb = sm.tile([128, KT * B_CORE], FP8, tag="st")   # s^T tiles, fp8
    hpre_sb = sm.tile([B_CORE, D], F32, tag="hpre")
    sq_sb = sm.tile([B_CORE, D], F32, tag="sq")
    y_sb = sm.tile([B_CORE, D], F32, tag="y")
    t2_sb = sm.tile([B_CORE, D], F32, tag="t2")
    out_sb = sm.tile([B_CORE, D], F32, tag="out")
    lnsr_sb = sm.tile([B_CORE, D], F32, tag="lnsr")
    lnbr_sb = sm.tile([B_CORE, D], F32, tag="lnbr")
    gmc_sb = sm.tile([B_CORE, 1], F32, tag="gmc")
    sum_h = [sm.tile([B_CORE, 1], F32, tag=f"sumh{h}", name=f"sumh{h}")
             for h in range(NH)]
    ssq_h = [sm.tile([B_CORE, 1], F32, tag=f"ssqh{h}", name=f"ssqh{h}")
             for h in range(NH)]
    sum_c = sm.tile([B_CORE, 1], F32, tag="sumc")
    m_c = sm.tile([B_CORE, 1], F32, tag="mc")
    msq_c = sm.tile([B_CORE, 1], F32, tag="msqc")
    ssq_c = sm.tile([B_CORE, 1], F32, tag="ssqc")
    var_c = sm.tile([B_CORE, 1], F32, tag="varc")
    std_c = sm.tile([B_CORE, 1], F32, tag="stdc")
    istd_c = sm.tile([B_CORE, 1], F32, tag="istdc")
    nmi_c = sm.tile([B_CORE, 1], F32, tag="nmic")
    eps_c = sm.tile([B_CORE, 1], F32, tag="epsc")
    warm_c = sm.tile([B_CORE, 1], F32, tag="warmc")

    # ---- three parallel DMA streams, ordered by first consumption:
    # sync: xt then vt/wt a-halves; scalar: activations then vt/wt
    # b-halves; gpsimd (SWDGE): bias then the whole ut matrix ----
    nc.sync.dma_start(out=xt_sb[:], in_=xt_d.ap())
    nc.scalar.dma_start(out=vt_sb[:, 4 * D:], in_=vt_d.ap()[:, 4 * D:])
    nc.scalar.dma_start(out=alt_sb[:], in_=alt_d.ap())
    nc.scalar.dma_start(out=al_sb[:], in_=al_d.ap())
    nc.sync.dma_start(out=vt_sb[:, :4 * D], in_=vt_d.ap()[:, :4 * D])
    nc.sync.dma_start(out=wt_sb[:, :4 * D], in_=wt_d.ap()[:, :4 * D])
    nc.scalar.dma_start(out=wt_sb[:, 4 * D:], in_=wt_d.ap()[:, 4 * D:])
    nc.gpsimd.dma_start(out=gmc_sb[:], in_=gm_d.ap().broadcast_to([B_CORE, 1]))
    nc.gpsimd.dma_start(out=bp_sb[:], in_=bp_d.ap())
    nc.gpsimd.dma_start(out=ut_sb[:, :4 * D], in_=ut_d.ap()[:, :4 * D])
    nc.gpsimd.dma_start(out=ut_sb[:, 4 * D:], in_=ut_d.ap()[:, 4 * D:])
    nc.scalar.dma_start(out=ha_sb[:], in_=ha_d.ap())
    if not ln_triv:
        nc.sync.dma_start(out=lnsr_sb[:],
                          in_=lns_d.ap().broadcast_to([B_CORE, D]))
        nc.scalar.dma_start(out=lnbr_sb[:],
                            in_=lnb_d.ap().broadcast_to([B_CORE, D]))

    nc.vector.memset(eps_c[:], 1e-5)

    if stage == "loads":
        nc.vector.tensor_copy(out_sb[:], ha_sb[:])
        nc.sync.dma_start(out=out_d.ap(), in_=out_sb[:])
        return

    # ---- PE warm-up: junk matmuls flip the HAM clock gate to 2.4GHz
    # before the real (DMA-gated) matmuls arrive ----
    warm_sb = sm.tile([128, 512], FP8, tag="warm")
    warm_ps = acc.tile([128, 512], F32, tag="warmps")
    nc.vector.memset(warm_sb[:], 1.0)
    for w in range(20):
        nc.tensor.matmul(warm_ps[:], warm_sb[:, :128], warm_sb[:],
                         start=True, stop=True)

    # ---- column-packed matmuls: the four M=32 accumulation chains
    # (t h=0, t h=1, h_T h=0, h_T h=1) run in distinct 32-column strips
    # of the PE array (tile_position), so they execute concurrently.
    # One [128, 512] PSUM bank holds all four as partition bands. ----
    tw = acc.tile([128, 512], F32, tag="tw")
    bnd = [tw[32 * c:32 * (c + 1), :] for c in range(4)]

    for h in range(NH):
        nc.tensor.matmul(bnd[2 + h], alt_sb[:],
                         bp_sb[:, 512 * h:512 * (h + 1)],
                         start=True, stop=False, tile_position=(0, 64 + 32 * h),
                         skip_group_check=True)
    for i in range(KT):
        xt_i = xt_sb[:, B_CORE * i:B_CORE * (i + 1)]
        for h in range(NH):
            nc.tensor.matmul(
                bnd[h], xt_i,
                vt_sb[:, D * i + 512 * h:D * i + 512 * (h + 1)],
                start=(i == 0), stop=(i == KT - 1),
                tile_position=(0, 32 * h), skip_group_check=True,
            )

    # s = t * repeat(alpha, R)  (reads the t bands at shifted partitions)
    for h in range(NH):
        o3 = s_sb[:, 512 * h:512 * (h + 1)].rearrange(
            "p (n r) -> p n r", r=R_RANK)
        i3 = bnd[h].rearrange("p (n r) -> p n r", r=R_RANK)
        a3 = al_sb[:, 32 * h:32 * (h + 1)].unsqueeze(-1).broadcast_to(
            [B_CORE, 32, R_RANK])
        nc.vector.tensor_mul(o3, i3, a3)
    for i in range(KT):
        xt_i = xt_sb[:, B_CORE * i:B_CORE * (i + 1)]
        for h in range(NH):
            nc.tensor.matmul(
                bnd[2 + h], xt_i,
                wt_sb[:, D * i + 512 * h:D * i + 512 * (h + 1)],
                start=False, stop=False, tile_position=(0, 64 + 32 * h),
                skip_group_check=True,
            )
    # preload both ACT tables (Square, Sqrt) before the LN tail needs them
    nc.scalar.activation(warm_c[:], eps_c[:],
                         mybir.ActivationFunctionType.Square)
    nc.scalar.activation(warm_c[:], eps_c[:], SQRT, bias=eps_c[:], scale=1.0)
    nc.scalar.activation(warm_c[:], eps_c[:],
                         mybir.ActivationFunctionType.Identity)

    if stage == "t":
        nc.sync.dma_start(out=out_d.ap(), in_=s_sb[:])
        return

    # ---- s^T tiles: one DVE 32x32 block-transpose, then 4 strided
    # partition-shifted copy-casts gather the blocks into fp8 [128, .] ----
    stt_sb = sm.tile([B_CORE, D], BF16, tag="stt")
    nc.vector.transpose(stt_sb[:], s_sb[:])
    for b in range(4):
        src_ap = stt_sb[:].rearrange(
            "p (j m) -> p j m", j=KT)[:, :, 32 * b:32 * b + B_CORE]
        dst = st_sb[32 * b:32 * (b + 1), :].rearrange(
            "p (j m) -> p j m", m=B_CORE)
        nc.vector.tensor_copy(dst, src_ap)

    # ---- h_T += s @ U^T (same column strips / bands as the W chain) ----
    for j in range(KT):
        st_j = st_sb[:, B_CORE * j:B_CORE * (j + 1)]
        for h in range(NH):
            nc.tensor.matmul(
                bnd[2 + h], st_j,
                ut_sb[:, D * j + 512 * h:D * j + 512 * (h + 1)],
                start=False, stop=(j == KT - 1),
                tile_position=(0, 64 + 32 * h), skip_group_check=True,
            )
    for h in range(NH):
        sl = slice(512 * h, 512 * (h + 1))
        # h_pre = (gamma/WSCALE) * h_T + h_A, with row-sums for the mean
        nc.vector.scalar_tensor_tensor(
            out=hpre_sb[:, sl], in0=bnd[2 + h], scalar=gmc_sb[:],
            in1=ha_sb[:, sl], op0=MULT, op1=ADD,
            accum_out=sum_h[h][:])
        # row-sums of squares on the Scalar engine (parallel to DVE)
        nc.scalar.activation(sq_sb[:, sl], hpre_sb[:, sl],
                             mybir.ActivationFunctionType.Square,
                             accum_out=ssq_h[h][:])

    if stage == "h":
        nc.sync.dma_start(out=out_d.ap(), in_=hpre_sb[:])
        return

    # ---- LayerNorm via E[x^2] - E[x]^2; istd in one ACT Rsqrt ----
    nc.vector.tensor_add(sum_c[:], sum_h[0][:], sum_h[1][:])
    nc.vector.tensor_add(ssq_c[:], ssq_h[0][:], ssq_h[1][:])
    nc.scalar.mul(m_c[:], sum_c[:], -1.0 / D)
    nc.vector.tensor_mul(msq_c[:], m_c[:], m_c[:])
    nc.vector.scalar_tensor_tensor(
        out=var_c[:], in0=ssq_c[:], scalar=1.0 / D, in1=msq_c[:],
        op0=MULT, op1=mybir.AluOpType.subtract)
    nc.scalar.activation(std_c[:], var_c[:], SQRT, bias=eps_c[:], scale=1.0)
    nc.vector.reciprocal(istd_c[:], std_c[:])
    nc.vector.tensor_mul(nmi_c[:], m_c[:], istd_c[:])
    # norm = hpre*istd - m*istd (ACT), out = norm*lns + lnb (DVE)
    for h in range(NH):
        sl = slice(512 * h, 512 * (h + 1))
        nc.scalar.activation(y_sb[:, sl], hpre_sb[:, sl],
                             mybir.ActivationFunctionType.Identity,
                             bias=nmi_c[:], scale=istd_c[:])
        if ln_triv:
            nc.sync.dma_start(out=out_d.ap()[:, sl], in_=y_sb[:, sl])
        else:
            nc.vector.tensor_mul(t2_sb[:, sl], y_sb[:, sl], lnsr_sb[:, sl])
            nc.vector.tensor_add(out_sb[:, sl], t2_sb[:, sl], lnbr_sb[:, sl])
            nc.sync.dma_start(out=out_d.ap()[:, sl], in_=out_sb[:, sl])


def _to_sbuf_layout(a):
    """[KT*128, F] logical -> [128, KT*F] partition-major."""
    f = a.shape[1]
    return np.ascontiguousarray(
        a.reshape(KT, 128, f).transpose(1, 0, 2).reshape(128, KT * f))


_FP8 = mybir.dt.np(FP8)
_BF16 = mybir.dt.np(BF16)


def _prep_in_maps(inputs):
    def f32c(x):
        return np.ascontiguousarray(np.asarray(x, dtype=np.float32))

    h_a = f32c(inputs["h_A"])
    alpha = f32c(inputs["alpha"])
    pool = np.asarray(inputs["pool_vectors"], dtype=np.float32)
    w_base = np.asarray(inputs["W_base"], dtype=np.float32)

    # pool_vectors rows: [U_n (D*R) | V_n (R*D) | bias_n (D)]
    u = pool[:, :D * R_RANK].reshape(N_EXP, D, R_RANK)
    v = pool[:, D * R_RANK:2 * D * R_RANK].reshape(N_EXP, R_RANK, D)
    bias_pool = pool[:, 2 * D * R_RANK:]                    # [64, D]
    bb = np.asarray(inputs["b_base"], dtype=np.float32).reshape(1, D)
    bp = np.ascontiguousarray(
        (np.concatenate([bias_pool, bb], axis=0) * WSCALE).astype(_BF16))
    ut = np.ascontiguousarray(_to_sbuf_layout(
        u.transpose(0, 2, 1).reshape(N_EXP * R_RANK, D)).astype(_FP8))
    vt = np.ascontiguousarray(_to_sbuf_layout(
        v.reshape(N_EXP * R_RANK, D).T * WSCALE).astype(_FP8))
    wt = np.ascontiguousarray(
        _to_sbuf_layout(w_base.T * WSCALE).astype(_FP8))
    lns = f32c(inputs["ln_scale"]).reshape(1, D)
    lnb = f32c(inputs["ln_bias"]).reshape(1, D)
    gm = f32c(inputs["gamma"]).reshape(1, 1) / WSCALE

    in_maps = []
    for k in range(N_CORES):
        rows = slice(B_CORE * k, B_CORE * (k + 1))
        ha_k = f32c(h_a[rows])
        al_k = f32c(alpha[rows])
        xt_k = np.ascontiguousarray(_to_sbuf_layout(ha_k.T).astype(_FP8))
        alt_k = np.ascontiguousarray(np.concatenate(
            [al_k.T, np.ones((1, B_CORE), np.float32)], axis=0).astype(_BF16))
        p = np.arange(B_CORE)[:, None]
        col = np.arange(4 * KT * B_CORE)[None, :]
        b_i, jm = col // (KT * B_CORE), col % (KT * B_CORE)
        exp_idx = (128 * (jm // B_CORE) + 32 * b_i + p) // R_RANK
        m_idx = np.broadcast_to(jm % B_CORE, exp_idx.shape)
        alr_k = np.ascontiguousarray(al_k[m_idx, exp_idx].astype(_BF16))
        in_maps.append({
            "ha": ha_k, "al": al_k, "alr": alr_k, "xt": xt_k, "alt": alt_k,
            "vt": vt, "ut": ut, "wt": wt, "bp": bp,
            "lns": lns, "lnb": lnb, "gm": gm,
        })
    return in_maps


def get_compiled(stage=None, ln_triv=False):
    key = (stage or STAGE, ln_triv)
    if key not in _COMPILED:
        _COMPILED[key] = _build(*key)
    return _COMPILED[key]


def _ln_trivial(inputs):
    return (np.allclose(np.asarray(inputs["ln_scale"], np.float32), 1.0)
            and np.allclose(np.asarray(inputs["ln_bias"], np.float32), 0.0))


def kernel(**inputs):
    nc = get_compiled(ln_triv=_ln_trivial(inputs))
    in_maps = _prep_in_maps(inputs)
    res = bass_utils.run_bass_kernel_spmd(
        nc, in_maps, core_ids=list(range(N_CORES)))
    return np.concatenate([r["out"] for r in res.results], axis=0)
